# revision 12
# baseline (speedup 1.0000x reference)
"""Trainium2 Bass kernel for nn_AblationAnomalyDetector (gnn_message_passing).

kernel(**inputs) -> (H [8192,8192] f32, ew [8192] f32)

8 NeuronCores, SPMD, node-dim sharded 1024 rows/core:
  Phase A (transposed layout [feature-part, row-free], fp32r matmuls):
    proj -> LN (stats via ones-matmuls) -> 2-token MHA (block-ones score
    matmuls + sigmoid softmax) -> fused^T [512,1024] + edge-weight MLP.
  AllGather fused^T (bf16) across cores -> [8*512, 1024] shared.
  Phase B per 128-row tile: bf16 matmul logits [128, 8192] -> per-chunk DVE
    max/max_index -> bit-encoded candidate top-16 (coarse val | col13) ->
    exact row max + softmax denominator via one ACT exp pass (accum_out) ->
    16 values scattered into u16 bit-planes via gpsimd local_scatter
    (zero-fill included) -> DMA to H^T.
Host: transposes/folds weights, shards inputs, concat + transpose output.
"""
import numpy as np

N = 8192
D = 512
NCORES = 8
R = N // NCORES          # 1024 rows per core
NT = R // 128            # 8 row-tiles per core
C = 16                   # scan chunks per row
CW = N // C              # 512
TOPK = 16
CH = 1022                # fp32 cols per local_scatter chunk
HCH = 5                  # ls-chunks per half (4 full + tail)
EPS = 1e-5

_compiled = None


def _build():
    import concourse.bass as bass
    import concourse.tile as tile
    import concourse.mybir as mybir
    from concourse import bacc

    fp32 = mybir.dt.float32
    fp32r = mybir.dt.float32r
    bf16 = mybir.dt.bfloat16
    u32 = mybir.dt.uint32
    i32 = mybir.dt.int32
    i16 = mybir.dt.int16
    u16 = mybir.dt.uint16
    AF = mybir.ActivationFunctionType
    OP = mybir.AluOpType
    ts, ds = bass.ts, bass.ds

    nc = bacc.Bacc("TRN2", target_bir_lowering=False, debug=False,
                   enable_asserts=True, num_devices=NCORES)

    def din(name, shape):
        return nc.dram_tensor(name, shape, fp32, kind="ExternalInput").ap()

    x0T_d = din("x0T", [D, R])
    x1T_d = din("x1T", [D, R])
    wp0T_d = din("wp0T", [D, D]); bp0_d = din("bp0", [1, D])
    wp1T_d = din("wp1T", [D, D]); bp1_d = din("bp1", [1, D])
    qkvT0_d = din("qkvT0", [D, 3 * D]); qkvb0_d = din("qkvb0", [1, 3 * D])
    qkvT1_d = din("qkvT1", [D, 3 * D]); qkvb1_d = din("qkvb1", [1, 3 * D])
    outT_d = din("outT", [D, D]); outb_d = din("outb", [1, D])
    ew1T_d = din("ew1T", [D, D // 2]); ewb1_d = din("ewb1", [1, D // 2])
    ew2T_d = din("ew2T", [D // 2, 1]); ewb2_d = din("ewb2", [1, 1])
    sco_d = din("sco", [D, 8])      # block 0.125
    exp8_d = din("exp8", [8, D])    # block ones

    ht_d = nc.dram_tensor("ht", [R, N], fp32, kind="ExternalOutput").ap()
    ew_d = nc.dram_tensor("ew", [1, R], fp32, kind="ExternalOutput").ap()

    agin = nc.dram_tensor("agin", [D, R], bf16)
    agout = nc.dram_tensor("agout", [NCORES * D, R], bf16, addr_space="Shared")

    with tile.TileContext(nc) as tc:
        with tc.tile_pool(name="pers", bufs=1) as pers:
            ones_f = pers.tile([1, R], fp32, tag="ones_f")
            nc.vector.memset(ones_f[:], 1.0)
            ones_r = pers.tile([1, R], fp32r, tag="ones_r")
            nc.vector.tensor_copy(ones_r[:], ones_f[:])
            fused_bf = pers.tile([128, 4, R], bf16, tag="fused_bf")

            _phase_a(nc, tc, bass, mybir, locals())
            nc.gpsimd.collective_compute(
                "AllGather", mybir.AluOpType.bypass,
                replica_groups=[list(range(NCORES))],
                ins=[agin.ap()], outs=[agout.ap()])
            _phase_b(nc, tc, bass, mybir, locals())

    nc.compile()
    return nc


def _phase_a(nc, tc, bass, mybir, env):
    fp32 = mybir.dt.float32
    fp32r = mybir.dt.float32r
    AF = mybir.ActivationFunctionType
    OP = mybir.AluOpType
    ts, ds = bass.ts, bass.ds
    ones_r = env["ones_r"]
    fused_bf = env["fused_bf"]
    agin = env["agin"]
    ew_d = env["ew_d"]
    RH = R // 2   # 512 rows per half

    with tc.tile_pool(name="aps", bufs=3, space="PSUM") as aps, \
         tc.tile_pool(name="stps", bufs=2, space="PSUM") as stps, \
         tc.tile_pool(name="astg", bufs=1) as astg, \
         tc.tile_pool(name="p_ctx", bufs=1) as p_ctx:

        def load_round(pool, dram_ap, kdim, fdim, tag):
            kc = max(kdim // 128, 1)
            p0 = min(kdim, 128)
            st = astg.tile([p0, kc, fdim], fp32, tag="stage")
            nc.sync.dma_start(st[:], dram_ap.rearrange("(c p) f -> p c f", p=p0))
            rt = pool.tile([p0, kc, fdim], fp32r, tag=tag)
            nc.vector.tensor_copy(rt[:], st[:])
            return rt

        def load_row_round(pool, dram_ap, fdim, tag):
            st = astg.tile([1, 1, fdim], fp32, tag="rowstage")
            nc.sync.dma_start(st[:, 0, :], dram_ap)
            rt = pool.tile([1, fdim], fp32r, tag=tag)
            nc.vector.tensor_copy(rt[:], st[:, 0, :])
            return rt

        ctx = p_ctx.tile([128, 4, R], fp32r, tag="ctx")

        for rh in range(2):
            rsl = ds(rh * RH, RH)   # row-half slice of R-sized free dims

            def mm_biash(psum, lhsT, brow, rhs, oc, nk=4):
                for kc in range(nk):
                    nc.tensor.matmul(psum[:], lhsT[:, kc, ts(oc, 128)],
                                     rhs[:, kc, :],
                                     start=(kc == 0), stop=False)
                nc.tensor.matmul(psum[:], brow[:, ts(oc, 128)],
                                 ones_r[:, rsl], start=False, stop=True)

            with tc.tile_pool(name=f"p_v{rh}", bufs=1) as p_v, \
                 tc.tile_pool(name=f"p_qk{rh}", bufs=1) as p_qk, \
                 tc.tile_pool(name=f"p_pn{rh}", bufs=1) as p_pn:
                # ---- stage 1: projections + LN -> pn0, pn1  [128, 4, RH]
                pn = {}
                for mi in (0, 1):
                    xd = env[f"x{mi}T_d"]
                    wd = env[f"wp{mi}T_d"]
                    bd = env[f"bp{mi}_d"]
                    with tc.tile_pool(name=f"s1m{rh}{mi}", bufs=1) as sm:
                        ones4 = sm.tile([128, 4, 1], fp32, tag="ones4")
                        nc.vector.memset(ones4[:], 1.0)
                        xstg = astg.tile([128, 4, RH], fp32, tag="xstage")
                        nc.sync.dma_start(
                            xstg[:],
                            xd.rearrange("(c p) f -> p c f", p=128)[:, :, rsl])
                        xT = sm.tile([128, 4, RH], fp32r, tag="xT")
                        nc.vector.tensor_copy(xT[:], xstg[:])
                        wpT = load_round(sm, wd, D, D, "wpT")
                        bp = load_row_round(sm, bd, D, "bp")
                        prelu = sm.tile([128, 4, RH], fp32, tag="prelu")
                        for oc in range(4):
                            ps = aps.tile([128, 512], fp32, tag="mm")
                            mm_biash(ps, wpT, bp, xT, oc)
                            nc.scalar.activation(prelu[:, oc, :], ps[:], AF.Relu)
                        psq = sm.tile([128, 4, RH], fp32, tag="tmp")
                        nc.scalar.activation(psq[:], prelu[:], AF.Square)
                        s1v = sm.tile([1, 2, RH], fp32, tag="s12")
                        for (si, src2) in ((0, prelu), (1, psq)):
                            sps = stps.tile([1, 512], fp32, tag="st")
                            for kc in range(4):
                                nc.tensor.matmul(sps[:], ones4[:, kc, :],
                                                 src2[:, kc, :],
                                                 start=(kc == 0), stop=(kc == 3))
                            nc.vector.tensor_copy(s1v[:, si, :], sps[:])
                        mean = sm.tile([1, RH], fp32, tag="mean")
                        nc.vector.tensor_scalar(mean[:], s1v[:, 0, :], 1.0 / D,
                                                None, op0=OP.mult)
                        var = sm.tile([1, RH], fp32, tag="var")
                        nc.vector.tensor_scalar(var[:], s1v[:, 1, :], 1.0 / D,
                                                None, op0=OP.mult)
                        msq = sm.tile([1, RH], fp32, tag="msq")
                        nc.vector.tensor_tensor(msq[:], mean[:], mean[:],
                                                op=OP.mult)
                        nc.vector.tensor_scalar(msq[:], msq[:], float(EPS), None,
                                                op0=OP.subtract)
                        nc.vector.tensor_tensor(var[:], var[:], msq[:],
                                                op=OP.subtract)
                        std = sm.tile([1, RH], fp32, tag="std")
                        nc.scalar.activation(std[:], var[:], AF.Sqrt)
                        istd = sm.tile([1, RH], fp32, tag="istd")
                        nc.vector.reciprocal(istd[:], std[:])
                        mean_b = sm.tile([128, RH], fp32, tag="meanb")
                        istd_b = sm.tile([128, RH], fp32, tag="istdb")
                        nc.gpsimd.partition_broadcast(mean_b[:], mean[:])
                        nc.gpsimd.partition_broadcast(istd_b[:], istd[:])
                        pnt = p_pn.tile([128, 4, RH], fp32r, tag=f"pn{mi}")
                        tmp = sm.tile([128, 4, RH], fp32, tag="tmp")
                        nc.vector.tensor_tensor(
                            tmp[:], prelu[:],
                            mean_b[:, None, :].to_broadcast([128, 4, RH]),
                            op=OP.subtract)
                        nc.vector.tensor_tensor(
                            pnt[:], tmp[:],
                            istd_b[:, None, :].to_broadcast([128, 4, RH]),
                            op=OP.mult)
                        pn[mi] = pnt

                # ---- stage 2: qkv -> qk + v
                qk = {}
                vv = {}
                for ti in (0, 1):
                    qd = env[f"qkvT{ti}_d"]
                    qb = env[f"qkvb{ti}_d"]
                    with tc.tile_pool(name=f"s2w{rh}{ti}", bufs=1) as sw:
                        qkvT = load_round(sw, qd, D, 3 * D, "qkvT")
                        qkvb = load_row_round(sw, qb, 3 * D, "qkvb")
                        qkt = p_qk.tile([128, 8, RH], fp32, tag=f"qk{ti}")
                        vt = p_v.tile([128, 4, RH], fp32, tag=f"v{ti}")
                        for oc in range(12):
                            dst = qkt[:, oc, :] if oc < 8 else vt[:, oc - 8, :]
                            ps = aps.tile([128, 512], fp32, tag="mm")
                            for kc in range(4):
                                nc.tensor.matmul(ps[:], qkvT[:, kc, ts(oc, 128)],
                                                 pn[ti][:, kc, :],
                                                 start=(kc == 0), stop=False)
                            nc.tensor.matmul(ps[:], qkvb[:, ts(oc, 128)],
                                             ones_r[:, rsl],
                                             start=False, stop=True)
                            nc.scalar.activation(dst[:], ps[:], AF.Copy)
                        qk[ti] = qkt
                        vv[ti] = vt

            # (p_pn closed) ---- stage 3: scores + attention weights
                # NOTE: p_pn still open here (same with-block); keep order:
                A = {}
                with tc.tile_pool(name=f"s3_{rh}", bufs=1) as s3:
                    sco = load_round(s3, env["sco_d"], D, 8, "sco")
                    s_t = {}
                    for (qi, ki) in ((0, 0), (0, 1), (1, 0), (1, 1)):
                        e = s3.tile([128, 4, RH], fp32r, tag="eprod")
                        nc.vector.tensor_tensor(e[:], qk[qi][:, 0:4, :],
                                                qk[ki][:, 4:8, :], op=OP.mult)
                        st_ = s3.tile([8, RH], fp32, tag=f"s{qi}{ki}")
                        sps = stps.tile([8, 512], fp32, tag="sc")
                        for kc in range(4):
                            nc.tensor.matmul(sps[:], sco[:, kc, :],
                                             e[:, kc, :],
                                             start=(kc == 0), stop=(kc == 3))
                        nc.vector.tensor_copy(st_[:], sps[:])
                        s_t[(qi, ki)] = st_
                    sig = {}
                    for qi in (0, 1):
                        dd = s3.tile([8, RH], fp32, tag=f"d{qi}")
                        nc.vector.tensor_tensor(dd[:], s_t[(qi, 0)][:],
                                                s_t[(qi, 1)][:], op=OP.subtract)
                        sg = s3.tile([8, RH], fp32, tag=f"sg{qi}")
                        nc.scalar.activation(sg[:], dd[:], AF.Sigmoid)
                        sig[qi] = sg
                    A0 = p_v.tile([8, RH], fp32r, tag="A0")
                    nc.vector.tensor_tensor(A0[:], sig[0][:], sig[1][:],
                                            op=OP.add)
                    A1 = p_v.tile([8, RH], fp32r, tag="A1")
                    nc.vector.tensor_scalar(A1[:], A0[:], -1.0, 2.0,
                                            op0=OP.mult, op1=OP.add)
                    A[0], A[1] = A0, A1

                # ---- stage 4: ctx half
                with tc.tile_pool(name=f"s4c{rh}", bufs=1) as s4c:
                    exp8 = load_round(s4c, env["exp8_d"], 8, D, "exp8")
                    ctmp = s4c.tile([128, 4, RH], fp32, tag="ctmp")
                    ctmp2 = s4c.tile([128, 4, RH], fp32, tag="ctmp2")
                    for (ai, dst) in ((0, ctmp), (1, ctmp2)):
                        Ae = s4c.tile([128, 4, RH], fp32, tag="Ae")
                        for oc in range(4):
                            ps = aps.tile([128, 512], fp32, tag="mm")
                            nc.tensor.matmul(ps[:], exp8[:, 0, ts(oc, 128)],
                                             A[ai][:],
                                             start=True, stop=True)
                            nc.scalar.activation(Ae[:, oc, :], ps[:], AF.Copy)
                        nc.vector.tensor_tensor(dst[:], Ae[:],
                                                vv[ai][:, 0:4, :], op=OP.mult)
                    nc.vector.tensor_tensor(ctx[:, :, rsl], ctmp[:], ctmp2[:],
                                            op=OP.add)

        # ---- stage 5: fused + edge weights (full R)
        with tc.tile_pool(name="s5", bufs=1) as s5:
            def mm_bias(psum, lhsT, brow, rhs, oc, rc, nk=4):
                for kc in range(nk):
                    nc.tensor.matmul(psum[:], lhsT[:, kc, ts(oc, 128)],
                                     rhs[:, kc, ts(rc, 512)],
                                     start=(kc == 0), stop=False)
                nc.tensor.matmul(psum[:], brow[:, ts(oc, 128)],
                                 ones_r[:, ts(rc, 512)], start=False, stop=True)

            outT = load_round(s5, env["outT_d"], D, D, "outT")
            outb = load_row_round(s5, env["outb_d"], D, "outb")
            fusedT_r = s5.tile([128, 4, R], fp32r, tag="fusedT_r")
            for oc in range(4):
                for rc in range(2):
                    ps = aps.tile([128, 512], fp32, tag="mm")
                    mm_bias(ps, outT, outb, ctx, oc, rc)
                    nc.scalar.activation(fusedT_r[:, oc, ts(rc, 512)], ps[:],
                                         AF.Copy)
            nc.vector.tensor_copy(fused_bf[:], fusedT_r[:])
            nc.sync.dma_start(agin.ap().rearrange("(c p) r -> p c r", p=128),
                              fused_bf[:])

            ew1T = load_round(s5, env["ew1T_d"], D, D // 2, "ew1T")
            ewb1 = load_row_round(s5, env["ewb1_d"], D // 2, "ewb1")
            ew2T = load_round(s5, env["ew2T_d"], D // 2, 1, "ew2T")
            ewb2 = load_row_round(s5, env["ewb2_d"], 1, "ewb2")
            hid = s5.tile([128, 2, R], fp32r, tag="hid")
            for oc in range(2):
                for rc in range(2):
                    ps = aps.tile([128, 512], fp32, tag="mm")
                    mm_bias(ps, ew1T, ewb1, fusedT_r, oc, rc)
                    nc.scalar.activation(hid[:, oc, ts(rc, 512)], ps[:], AF.Relu)
            ew_sb = s5.tile([1, R], fp32, tag="ew_sb")
            for rc in range(2):
                sps = stps.tile([1, 512], fp32, tag="st")
                for kc in range(2):
                    nc.tensor.matmul(sps[:], ew2T[:, kc, :],
                                     hid[:, kc, ts(rc, 512)],
                                     start=(kc == 0), stop=False)
                nc.tensor.matmul(sps[:], ewb2[:], ones_r[:, ts(rc, 512)],
                                 start=False, stop=True)
                nc.scalar.activation(ew_sb[:, ts(rc, 512)], sps[:], AF.Sigmoid)
            ewm = s5.tile([1, R], fp32, tag="ewm")
            nc.vector.tensor_scalar(ewm[:], ew_sb[:], 1e-8, None, op0=OP.max)
            nc.sync.dma_start(ew_d, ewm[:])


def _phase_b(nc, tc, bass, mybir, env):
    fp32 = mybir.dt.float32
    bf16 = mybir.dt.bfloat16
    u32 = mybir.dt.uint32
    i32 = mybir.dt.int32
    i16 = mybir.dt.int16
    u16 = mybir.dt.uint16
    AF = mybir.ActivationFunctionType
    OP = mybir.AluOpType
    ts, ds = bass.ts, bass.ds
    fused_bf = env["fused_bf"]
    agout = env["agout"]
    ht_d = env["ht_d"]

    with tc.tile_pool(name="brhs", bufs=1) as brhs, \
         tc.tile_pool(name="blog", bufs=2) as blog, \
         tc.tile_pool(name="bsc", bufs=2) as bsc, \
         tc.tile_pool(name="bw", bufs=2) as bw, \
         tc.tile_pool(name="bps", bufs=6, space="PSUM") as bps:

        rhs = brhs.tile([128, 4, NCORES, R], bf16, tag="rhs")
        for s in range(NCORES):
            nc.sync.dma_start(
                rhs[:, :, s, :],
                agout.ap()[s * D:(s + 1) * D, :].rearrange(
                    "(c p) r -> p c r", p=128))

        lsgrid = brhs.tile([128, 2 * HCH, 1], i32, tag="lsgrid")
        nc.gpsimd.iota(lsgrid[:, 0:HCH, :], pattern=[[2 * CH, HCH], [0, 1]],
                       base=0, channel_multiplier=0)
        nc.gpsimd.iota(lsgrid[:, HCH:2 * HCH, :], pattern=[[2 * CH, HCH], [0, 1]],
                       base=2 * 4096, channel_multiplier=0)
        cgrid = brhs.tile([128, C, 1], u32, tag="cgrid")
        nc.gpsimd.iota(cgrid[:], pattern=[[CW, C], [0, 1]], base=0,
                       channel_multiplier=0)

        for t in range(NT):
            logits = blog.tile([128, C, CW], fp32, tag="logits")
            for cb in range(C):
                ps = bps.tile([128, 512], fp32, tag="pb")
                s, half = cb // 2, cb % 2
                for kc in range(4):
                    nc.tensor.matmul(ps[:],
                                     fused_bf[:, kc, ds(t * 128, 128)],
                                     rhs[:, kc, s, ts(half, 512)],
                                     start=(kc == 0), stop=(kc == 3))
                nc.scalar.activation(logits[:, cb, :], ps[:], AF.Copy)

            cmax = bw.tile([128, C, 8], fp32, tag="cmax")
            cidx = bw.tile([128, C, 8], u32, tag="cidx")
            for cb in range(C):
                nc.vector.max(cmax[:, cb, :], logits[:, cb, :])
                nc.vector.max_index(cidx[:, cb, :], cmax[:, cb, :],
                                    logits[:, cb, :])
            m8 = bw.tile([128, 8], fp32, tag="m8")
            nc.vector.max(m8[:], cmax[:].rearrange("p c e -> p (c e)"))
            negM = bw.tile([128, 1], fp32, tag="negM")
            nc.vector.tensor_scalar(negM[:], m8[:, 0:1], -1.0, None, op0=OP.mult)

            colf = bw.tile([128, C, 8], u32, tag="colf")
            nc.vector.tensor_tensor(colf[:], cidx[:],
                                    cgrid[:].to_broadcast([128, C, 8]), op=OP.add)
            enc = bw.tile([128, C * 8], fp32, tag="enc")
            nc.vector.tensor_scalar(enc[:].bitcast(u32),
                                    cmax[:].rearrange("p c e -> p (c e)").bitcast(u32),
                                    0xFFFFE000, None, op0=OP.bitwise_and)
            nc.vector.tensor_tensor(enc[:].bitcast(u32), enc[:].bitcast(u32),
                                    colf[:].rearrange("p c e -> p (c e)"),
                                    op=OP.bitwise_or)
            top16 = bw.tile([128, TOPK], fp32, tag="top16")
            nc.vector.max(top16[:, 0:8], enc[:])
            enc2 = bw.tile([128, C * 8], fp32, tag="enc2")
            nc.vector.match_replace(out=enc2[:], in_to_replace=top16[:, 0:8],
                                    in_values=enc[:], imm_value=-3.0e38)
            nc.vector.max(top16[:, 8:16], enc2[:])

            cols = bw.tile([128, TOPK], u32, tag="cols")
            nc.vector.tensor_scalar(cols[:], top16[:].bitcast(u32), 0x1FFF,
                                    None, op0=OP.bitwise_and)
            lvc = bw.tile([128, TOPK], fp32, tag="lvc")
            nc.vector.tensor_scalar(lvc[:].bitcast(u32), top16[:].bitcast(u32),
                                    0xFFFFE000, None, op0=OP.bitwise_and)
            nc.vector.tensor_copy(lvc[:, 0:1], m8[:, 0:1])

            den = bw.tile([128, 1], fp32, tag="den")
            nc.scalar.activation(logits[:].rearrange("p c w -> p (c w)"),
                                 logits[:].rearrange("p c w -> p (c w)"),
                                 AF.Exp, bias=negM[:], scale=1.0,
                                 accum_out=den[:])
            recip = bw.tile([128, 1], fp32, tag="recip")
            nc.vector.reciprocal(recip[:], den[:])

            evals = bw.tile([128, TOPK], fp32, tag="evals")
            nc.scalar.activation(evals[:], lvc[:], AF.Exp, bias=negM[:])
            vals = bw.tile([128, TOPK], fp32, tag="vals")
            nc.vector.tensor_scalar(vals[:], evals[:], recip[:], None,
                                    op0=OP.mult)

            col2 = bw.tile([128, TOPK, 2], i32, tag="col2")
            nc.vector.tensor_scalar(col2[:, :, 0:1], cols[:, :, None], 2.0,
                                    None, op0=OP.mult)
            nc.vector.tensor_scalar(col2[:, :, 1:2], cols[:, :, None], 2.0,
                                    1.0, op0=OP.mult, op1=OP.add)
            shifted = bw.tile([128, 2 * HCH, 2 * TOPK], i32, tag="shifted")
            nc.vector.tensor_tensor(
                shifted[:],
                col2[:].rearrange("p k two -> p (k two)")[:, None, :]
                    .to_broadcast([128, 2 * HCH, 2 * TOPK]),
                lsgrid[:].to_broadcast([128, 2 * HCH, 2 * TOPK]),
                op=OP.subtract)
            oob = bw.tile([128, 2 * HCH, 2 * TOPK], i32, tag="oob")
            nc.vector.tensor_scalar(oob[:], shifted[:], float(2 * CH), -32768.0,
                                    op0=OP.is_ge, op1=OP.mult)
            nc.vector.tensor_tensor(shifted[:], shifted[:], oob[:], op=OP.add)
            idx16 = bw.tile([128, 2 * HCH, 2 * TOPK], i16, tag="idx16")
            nc.vector.tensor_copy(idx16[:], shifted[:])

            data16 = vals[:].bitcast(u16)
            for h in range(2):
                outu = bsc.tile([128, 8192], u16, tag="outu")
                for c in range(HCH):
                    ne = 2 * CH if c < HCH - 1 else 8192 - 2 * CH * (HCH - 1)
                    nc.gpsimd.local_scatter(
                        outu[:, 2 * CH * c: 2 * CH * c + ne],
                        data16,
                        idx16[:, h * HCH + c, :],
                        channels=128, num_elems=ne, num_idxs=2 * TOPK)
                nc.sync.dma_start(ht_d[ts(t, 128), ts(h, 4096)],
                                  outu[:].bitcast(fp32))


def _host_prep(inputs):
    f = np.float32
    x0 = np.asarray(inputs["x0"], f); x1 = np.asarray(inputs["x1"], f)
    g0 = np.asarray(inputs["g0"], f); beta0 = np.asarray(inputs["beta0"], f)
    g1 = np.asarray(inputs["g1"], f); beta1 = np.asarray(inputs["beta1"], f)
    in_w = np.asarray(inputs["in_w"], f); in_b = np.asarray(inputs["in_b"], f)
    out_w = np.asarray(inputs["out_w"], f); out_b = np.asarray(inputs["out_b"], f)
    ew_w1 = np.asarray(inputs["ew_w1"], f); ew_b1 = np.asarray(inputs["ew_b1"], f)
    ew_w2 = np.asarray(inputs["ew_w2"], f); ew_b2 = np.asarray(inputs["ew_b2"], f)

    shared = {
        "wp0T": np.ascontiguousarray(np.asarray(inputs["w_p0"], f).T),
        "bp0": np.asarray(inputs["b_p0"], f)[None, :],
        "wp1T": np.ascontiguousarray(np.asarray(inputs["w_p1"], f).T),
        "bp1": np.asarray(inputs["b_p1"], f)[None, :],
        "qkvT0": np.ascontiguousarray((in_w * g0[None, :]).T),
        "qkvb0": (in_w @ beta0 + in_b)[None, :],
        "qkvT1": np.ascontiguousarray((in_w * g1[None, :]).T),
        "qkvb1": (in_w @ beta1 + in_b)[None, :],
        "outT": np.ascontiguousarray((0.5 * out_w).T),
        "outb": out_b[None, :],
        "ew1T": np.ascontiguousarray(ew_w1.T),
        "ewb1": ew_b1[None, :],
        "ew2T": np.ascontiguousarray(ew_w2.T),
        "ewb2": ew_b2[None, :],
    }
    sco = np.zeros((D, 8), f)
    for h in range(8):
        sco[h * 64:(h + 1) * 64, h] = 0.125
    shared["sco"] = sco
    shared["exp8"] = np.ascontiguousarray((sco.T != 0)).astype(f)

    x0T = np.ascontiguousarray(x0.T)
    x1T = np.ascontiguousarray(x1.T)
    maps = []
    for c in range(NCORES):
        m = dict(shared)
        m["x0T"] = np.ascontiguousarray(x0T[:, c * R:(c + 1) * R])
        m["x1T"] = np.ascontiguousarray(x1T[:, c * R:(c + 1) * R])
        maps.append(m)
    return maps


LAST_EXEC_NS = None


def kernel(**inputs):
    global _compiled, LAST_EXEC_NS
    import os
    from concourse.bass_utils import run_bass_kernel_spmd
    if _compiled is None:
        _compiled = _build()
    maps = _host_prep(inputs)
    trace = bool(os.environ.get("KERNEL_TRACE"))
    res = run_bass_kernel_spmd(_compiled, maps, core_ids=list(range(NCORES)),
                               trace=trace)
    LAST_EXEC_NS = res.exec_time_ns
    ht = np.concatenate([res.results[c]["ht"] for c in range(NCORES)], axis=0)
    H = np.ascontiguousarray(ht.T)
    ew = np.concatenate([res.results[c]["ew"][0] for c in range(NCORES)])
    return H, ew


# revision 14
# speedup vs baseline: 1.1943x; 1.1943x over previous
"""Trainium2 Bass kernel for nn_AblationAnomalyDetector (gnn_message_passing).

kernel(**inputs) -> (H [8192,8192] f32, ew [8192] f32)

8 NeuronCores, SPMD, node-dim sharded 1024 rows/core, fp16 matmuls:
  Phase A (transposed layout [feature-part, row-free]): proj -> LN (stats via
  ones-matmuls, biases via ACT per-partition bias APs) -> 2-token MHA ->
  fused^T [512,1024] fp16 + edge-weight MLP.
  AllGather fused^T (fp16) -> [8*512, 1024] shared.
  Phase B per 128-row tile: fp16 matmul logits [128, 8192] -> per-chunk DVE
  max/max_index -> bit-encoded candidate top-16 -> exact row max + softmax
  denominator via one in-place ACT exp pass (accum_out) -> 16 values
  scattered into u16 bit-planes via gpsimd local_scatter -> DMA to H^T.
"""
import numpy as np

N = 8192
D = 512
NCORES = 8
R = N // NCORES          # 1024 rows per core
NT = R // 128            # 8 row-tiles per core
C = 16                   # scan chunks per row
CW = N // C              # 512
TOPK = 16
CH = 1022                # fp32 cols per local_scatter chunk
HCH = 5                  # ls-chunks per half (4 full + tail)
EPS = 1e-5

_compiled = None
LAST_EXEC_NS = None


def _build():
    import concourse.bass as bass
    import concourse.tile as tile
    import concourse.mybir as mybir
    from concourse import bacc

    fp32 = mybir.dt.float32
    fp16 = mybir.dt.float16

    nc = bacc.Bacc("TRN2", target_bir_lowering=False, debug=False,
                   enable_asserts=True, num_devices=NCORES)

    def din16(name, shape):
        return nc.dram_tensor(name, shape, fp16, kind="ExternalInput").ap()

    def din32(name, shape):
        return nc.dram_tensor(name, shape, fp32, kind="ExternalInput").ap()

    env = {}
    env["x0T_d"] = din16("x0T", [D, R])
    env["x1T_d"] = din16("x1T", [D, R])
    env["wp0T_d"] = din16("wp0T", [D, D]); env["bp0_d"] = din32("bp0", [128, 4])
    env["wp1T_d"] = din16("wp1T", [D, D]); env["bp1_d"] = din32("bp1", [128, 4])
    env["qkvT0_d"] = din16("qkvT0", [D, 3 * D])
    env["qkvb0_d"] = din32("qkvb0", [128, 12])
    env["qkvT1_d"] = din16("qkvT1", [D, 3 * D])
    env["qkvb1_d"] = din32("qkvb1", [128, 12])
    env["outT_d"] = din16("outT", [D, D]); env["outb_d"] = din32("outb", [128, 4])
    env["ew1T_d"] = din16("ew1T", [D, D // 2])
    env["ewb1_d"] = din32("ewb1", [128, 2])
    env["ew2T_d"] = din16("ew2T", [D // 2, 1])
    env["ewb2_d"] = din32("ewb2", [1, 1])
    env["sco_d"] = din16("sco", [D, 8])
    env["exp8_d"] = din16("exp8", [8, D])

    env["ht_d"] = nc.dram_tensor("ht", [R, N], fp32, kind="ExternalOutput").ap()
    env["ew_d"] = nc.dram_tensor("ew", [1, R], fp32, kind="ExternalOutput").ap()

    env["agin"] = nc.dram_tensor("agin", [D, R], fp16)
    env["agout"] = nc.dram_tensor("agout", [NCORES * D, R], fp16,
                                  addr_space="Shared")

    with tile.TileContext(nc) as tc:
        with tc.tile_pool(name="pers", bufs=1) as pers:
            fused16 = pers.tile([128, 4, R], fp16, tag="fused16")
            env["fused16"] = fused16
            _phase_a(nc, tc, bass, mybir, env)
            nc.gpsimd.collective_compute(
                "AllGather", mybir.AluOpType.bypass,
                replica_groups=[list(range(NCORES))],
                ins=[env["agin"].ap()], outs=[env["agout"].ap()])
            _phase_b(nc, tc, bass, mybir, env)

    nc.compile()
    return nc


def _phase_a(nc, tc, bass, mybir, env):
    fp32 = mybir.dt.float32
    fp16 = mybir.dt.float16
    AF = mybir.ActivationFunctionType
    OP = mybir.AluOpType
    ts, ds = bass.ts, bass.ds
    fused16 = env["fused16"]
    RH = R // 2

    with tc.tile_pool(name="aps", bufs=4, space="PSUM") as aps, \
         tc.tile_pool(name="stps", bufs=2, space="PSUM") as stps, \
         tc.tile_pool(name="p_ctx", bufs=1) as p_ctx:

        def load16(pool, dram_ap, kdim, fdim, tag):
            kc = max(kdim // 128, 1)
            p0 = min(kdim, 128)
            t = pool.tile([p0, kc, fdim], fp16, tag=tag)
            nc.sync.dma_start(t[:], dram_ap.rearrange("(c p) f -> p c f", p=p0))
            return t

        def loadb(pool, dram_ap, ncols, tag):
            t = pool.tile([dram_ap.shape[0], ncols], fp32, tag=tag)
            nc.sync.dma_start(t[:], dram_ap)
            return t

        ctx = p_ctx.tile([128, 4, R], fp16, tag="ctx")

        for rh in range(2):
            rsl = ds(rh * RH, RH)

            with tc.tile_pool(name=f"p_v{rh}", bufs=1) as p_v, \
                 tc.tile_pool(name=f"p_qk{rh}", bufs=1) as p_qk, \
                 tc.tile_pool(name=f"p_pn{rh}", bufs=1) as p_pn:
                # ---- stage 1: projections + LN
                pn = {}
                for mi in (0, 1):
                    with tc.tile_pool(name=f"s1m{rh}{mi}", bufs=1) as sm:
                        ones4 = sm.tile([128, 4, 1], fp16, tag="ones4")
                        nc.vector.memset(ones4[:], 1.0)
                        xT = sm.tile([128, 4, RH], fp16, tag="xT")
                        nc.sync.dma_start(
                            xT[:],
                            env[f"x{mi}T_d"].rearrange(
                                "(c p) f -> p c f", p=128)[:, :, rsl])
                        wpT = load16(sm, env[f"wp{mi}T_d"], D, D, "wpT")
                        bp = loadb(sm, env[f"bp{mi}_d"], 4, "bp")
                        prelu = sm.tile([128, 4, RH], fp16, tag="prelu")
                        for oc in range(4):
                            ps = aps.tile([128, 512], fp32, tag="mm")
                            for kc in range(4):
                                nc.tensor.matmul(ps[:], wpT[:, kc, ts(oc, 128)],
                                                 xT[:, kc, :],
                                                 start=(kc == 0), stop=(kc == 3))
                            nc.scalar.activation(prelu[:, oc, :], ps[:], AF.Relu,
                                                 bias=bp[:, oc:oc + 1])
                        psq = sm.tile([128, 4, RH], fp16, tag="tmp16")
                        nc.scalar.activation(psq[:], prelu[:], AF.Square)
                        s1v = sm.tile([1, 2, RH], fp32, tag="s12")
                        for (si, src2) in ((0, prelu), (1, psq)):
                            sps = stps.tile([1, 512], fp32, tag="st")
                            for kc in range(4):
                                nc.tensor.matmul(sps[:], ones4[:, kc, :],
                                                 src2[:, kc, :],
                                                 start=(kc == 0), stop=(kc == 3))
                            nc.vector.tensor_copy(s1v[:, si, :], sps[:])
                        mean = sm.tile([1, RH], fp32, tag="mean")
                        nc.vector.tensor_scalar(mean[:], s1v[:, 0, :], 1.0 / D,
                                                None, op0=OP.mult)
                        var = sm.tile([1, RH], fp32, tag="var")
                        nc.vector.tensor_scalar(var[:], s1v[:, 1, :], 1.0 / D,
                                                None, op0=OP.mult)
                        msq = sm.tile([1, RH], fp32, tag="msq")
                        nc.vector.tensor_tensor(msq[:], mean[:], mean[:],
                                                op=OP.mult)
                        nc.vector.tensor_scalar(msq[:], msq[:], float(EPS), None,
                                                op0=OP.subtract)
                        nc.vector.tensor_tensor(var[:], var[:], msq[:],
                                                op=OP.subtract)
                        std = sm.tile([1, RH], fp32, tag="std")
                        nc.scalar.activation(std[:], var[:], AF.Sqrt)
                        istd = sm.tile([1, RH], fp32, tag="istd")
                        nc.vector.reciprocal(istd[:], std[:])
                        mean_b = sm.tile([128, RH], fp32, tag="meanb")
                        istd_b = sm.tile([128, RH], fp32, tag="istdb")
                        nc.gpsimd.partition_broadcast(mean_b[:], mean[:])
                        nc.gpsimd.partition_broadcast(istd_b[:], istd[:])
                        pnt = p_pn.tile([128, 4, RH], fp16, tag=f"pn{mi}")
                        tmp = sm.tile([128, 4, RH], fp32, tag="tmp32")
                        nc.vector.tensor_tensor(
                            tmp[:], prelu[:],
                            mean_b[:, None, :].to_broadcast([128, 4, RH]),
                            op=OP.subtract)
                        nc.vector.tensor_tensor(
                            pnt[:], tmp[:],
                            istd_b[:, None, :].to_broadcast([128, 4, RH]),
                            op=OP.mult)
                        pn[mi] = pnt

                # ---- stage 2: qkv
                qk = {}
                vv = {}
                for ti in (0, 1):
                    with tc.tile_pool(name=f"s2w{rh}{ti}", bufs=1) as sw:
                        qkvT = load16(sw, env[f"qkvT{ti}_d"], D, 3 * D, "qkvT")
                        qkvb = loadb(sw, env[f"qkvb{ti}_d"], 12, "qkvb")
                        qkt = p_qk.tile([128, 8, RH], fp16, tag=f"qk{ti}")
                        vt = p_v.tile([128, 4, RH], fp16, tag=f"v{ti}")
                        for oc in range(12):
                            dst = qkt[:, oc, :] if oc < 8 else vt[:, oc - 8, :]
                            ps = aps.tile([128, 512], fp32, tag="mm")
                            for kc in range(4):
                                nc.tensor.matmul(ps[:], qkvT[:, kc, ts(oc, 128)],
                                                 pn[ti][:, kc, :],
                                                 start=(kc == 0), stop=(kc == 3))
                            nc.scalar.activation(dst[:], ps[:], AF.Identity,
                                                 bias=qkvb[:, oc:oc + 1])
                        qk[ti] = qkt
                        vv[ti] = vt

                # ---- stage 3: scores + attention weights
                A = {}
                with tc.tile_pool(name=f"s3_{rh}", bufs=1) as s3:
                    sco = load16(s3, env["sco_d"], D, 8, "sco")
                    s_t = {}
                    for (qi, ki) in ((0, 0), (0, 1), (1, 0), (1, 1)):
                        e = s3.tile([128, 4, RH], fp16, tag="eprod")
                        nc.vector.tensor_tensor(e[:], qk[qi][:, 0:4, :],
                                                qk[ki][:, 4:8, :], op=OP.mult)
                        st_ = s3.tile([8, RH], fp32, tag=f"s{qi}{ki}")
                        sps = stps.tile([8, 512], fp32, tag="sc")
                        for kc in range(4):
                            nc.tensor.matmul(sps[:], sco[:, kc, :], e[:, kc, :],
                                             start=(kc == 0), stop=(kc == 3))
                        nc.vector.tensor_copy(st_[:], sps[:])
                        s_t[(qi, ki)] = st_
                    sig = {}
                    for qi in (0, 1):
                        dd = s3.tile([8, RH], fp32, tag=f"d{qi}")
                        nc.vector.tensor_tensor(dd[:], s_t[(qi, 0)][:],
                                                s_t[(qi, 1)][:], op=OP.subtract)
                        sg = s3.tile([8, RH], fp32, tag=f"sg{qi}")
                        nc.scalar.activation(sg[:], dd[:], AF.Sigmoid)
                        sig[qi] = sg
                    A0 = p_v.tile([8, RH], fp16, tag="A0")
                    nc.vector.tensor_tensor(A0[:], sig[0][:], sig[1][:],
                                            op=OP.add)
                    A1 = p_v.tile([8, RH], fp16, tag="A1")
                    nc.vector.tensor_scalar(A1[:], A0[:], -1.0, 2.0,
                                            op0=OP.mult, op1=OP.add)
                    A[0], A[1] = A0, A1

                # ---- stage 4: ctx half
                with tc.tile_pool(name=f"s4c{rh}", bufs=1) as s4c:
                    exp8 = load16(s4c, env["exp8_d"], 8, D, "exp8")
                    ctmp = s4c.tile([128, 4, RH], fp16, tag="ctmp")
                    ctmp2 = s4c.tile([128, 4, RH], fp16, tag="ctmp2")
                    for (ai, dst) in ((0, ctmp), (1, ctmp2)):
                        Ae = s4c.tile([128, 4, RH], fp16, tag="Ae")
                        for oc in range(4):
                            ps = aps.tile([128, 512], fp32, tag="mm")
                            nc.tensor.matmul(ps[:], exp8[:, 0, ts(oc, 128)],
                                             A[ai][:], start=True, stop=True)
                            nc.scalar.activation(Ae[:, oc, :], ps[:], AF.Copy)
                        nc.vector.tensor_tensor(dst[:], Ae[:],
                                                vv[ai][:, 0:4, :], op=OP.mult)
                    nc.vector.tensor_tensor(ctx[:, :, rsl], ctmp[:], ctmp2[:],
                                            op=OP.add)

        # ---- stage 5: fused + edge weights (full R)
        with tc.tile_pool(name="s5", bufs=1) as s5:
            outT = load16(s5, env["outT_d"], D, D, "outT")
            outb = loadb(s5, env["outb_d"], 4, "outb")
            for oc in range(4):
                for rc in range(2):
                    ps = aps.tile([128, 512], fp32, tag="mm")
                    for kc in range(4):
                        nc.tensor.matmul(ps[:], outT[:, kc, ts(oc, 128)],
                                         ctx[:, kc, ts(rc, 512)],
                                         start=(kc == 0), stop=(kc == 3))
                    nc.scalar.activation(fused16[:, oc, ts(rc, 512)], ps[:],
                                         AF.Identity, bias=outb[:, oc:oc + 1])
            nc.sync.dma_start(
                env["agin"].ap().rearrange("(c p) r -> p c r", p=128),
                fused16[:])

            ew1T = load16(s5, env["ew1T_d"], D, D // 2, "ew1T")
            ewb1 = loadb(s5, env["ewb1_d"], 2, "ewb1")
            ew2T = load16(s5, env["ew2T_d"], D // 2, 1, "ew2T")
            ewb2 = loadb(s5, env["ewb2_d"], 1, "ewb2")
            hid = s5.tile([128, 2, R], fp16, tag="hid")
            for oc in range(2):
                for rc in range(2):
                    ps = aps.tile([128, 512], fp32, tag="mm")
                    for kc in range(4):
                        nc.tensor.matmul(ps[:], ew1T[:, kc, ts(oc, 128)],
                                         fused16[:, kc, ts(rc, 512)],
                                         start=(kc == 0), stop=(kc == 3))
                    nc.scalar.activation(hid[:, oc, ts(rc, 512)], ps[:], AF.Relu,
                                         bias=ewb1[:, oc:oc + 1])
            ew_sb = s5.tile([1, R], fp32, tag="ew_sb")
            for rc in range(2):
                sps = stps.tile([1, 512], fp32, tag="st")
                for kc in range(2):
                    nc.tensor.matmul(sps[:], ew2T[:, kc, :],
                                     hid[:, kc, ts(rc, 512)],
                                     start=(kc == 0), stop=(kc == 1))
                nc.scalar.activation(ew_sb[:, ts(rc, 512)], sps[:], AF.Sigmoid,
                                     bias=ewb2[:, 0:1])
            ewm = s5.tile([1, R], fp32, tag="ewm")
            nc.vector.tensor_scalar(ewm[:], ew_sb[:], 1e-8, None, op0=OP.max)
            nc.sync.dma_start(env["ew_d"], ewm[:])


def _phase_b(nc, tc, bass, mybir, env):
    fp32 = mybir.dt.float32
    fp16 = mybir.dt.float16
    u32 = mybir.dt.uint32
    i32 = mybir.dt.int32
    i16 = mybir.dt.int16
    u16 = mybir.dt.uint16
    AF = mybir.ActivationFunctionType
    OP = mybir.AluOpType
    ts, ds = bass.ts, bass.ds
    fused16 = env["fused16"]
    agout = env["agout"]
    ht_d = env["ht_d"]

    with tc.tile_pool(name="brhs", bufs=1) as brhs, \
         tc.tile_pool(name="blog", bufs=2) as blog, \
         tc.tile_pool(name="bsc", bufs=2) as bsc, \
         tc.tile_pool(name="bw", bufs=2) as bw, \
         tc.tile_pool(name="bps", bufs=8, space="PSUM") as bps:

        rhs = brhs.tile([128, 4, NCORES, R], fp16, tag="rhs")
        for s in range(NCORES):
            nc.sync.dma_start(
                rhs[:, :, s, :],
                agout.ap()[s * D:(s + 1) * D, :].rearrange(
                    "(c p) r -> p c r", p=128))

        lsgrid = brhs.tile([128, 2 * HCH, 1], i32, tag="lsgrid")
        nc.gpsimd.iota(lsgrid[:, 0:HCH, :], pattern=[[2 * CH, HCH], [0, 1]],
                       base=0, channel_multiplier=0)
        nc.gpsimd.iota(lsgrid[:, HCH:2 * HCH, :], pattern=[[2 * CH, HCH], [0, 1]],
                       base=2 * 4096, channel_multiplier=0)
        cgrid = brhs.tile([128, C, 1], u32, tag="cgrid")
        nc.gpsimd.iota(cgrid[:], pattern=[[CW, C], [0, 1]], base=0,
                       channel_multiplier=0)

        for t in range(NT):
            logits = blog.tile([128, C, CW], fp32, tag="logits")
            for g in range(2):
                pss = [bps.tile([128, 512], fp32, tag="pb", name=f"pb{j}") for j in range(8)]
                for kc in range(4):
                    for j in range(8):
                        cb = g * 8 + j
                        s, half = cb // 2, cb % 2
                        nc.tensor.matmul(pss[j][:],
                                         fused16[:, kc, ds(t * 128, 128)],
                                         rhs[:, kc, s, ts(half, 512)],
                                         start=(kc == 0), stop=(kc == 3))
                for j in range(8):
                    cb = g * 8 + j
                    nc.scalar.activation(logits[:, cb, :], pss[j][:], AF.Copy)

            cmax = bw.tile([128, C, 8], fp32, tag="cmax")
            cidx = bw.tile([128, C, 8], u32, tag="cidx")
            for cb in range(C):
                nc.vector.max(cmax[:, cb, :], logits[:, cb, :])
                nc.vector.max_index(cidx[:, cb, :], cmax[:, cb, :],
                                    logits[:, cb, :])
            m8 = bw.tile([128, 8], fp32, tag="m8")
            nc.vector.max(m8[:], cmax[:].rearrange("p c e -> p (c e)"))
            negM = bw.tile([128, 1], fp32, tag="negM")
            nc.vector.tensor_scalar(negM[:], m8[:, 0:1], -1.0, None, op0=OP.mult)

            colf = bw.tile([128, C, 8], u32, tag="colf")
            nc.vector.tensor_tensor(colf[:], cidx[:],
                                    cgrid[:].to_broadcast([128, C, 8]), op=OP.add)
            enc = bw.tile([128, C * 8], fp32, tag="enc")
            nc.vector.tensor_scalar(enc[:].bitcast(u32),
                                    cmax[:].rearrange("p c e -> p (c e)").bitcast(u32),
                                    0xFFFFE000, None, op0=OP.bitwise_and)
            nc.vector.tensor_tensor(enc[:].bitcast(u32), enc[:].bitcast(u32),
                                    colf[:].rearrange("p c e -> p (c e)"),
                                    op=OP.bitwise_or)
            top16 = bw.tile([128, TOPK], fp32, tag="top16")
            nc.vector.max(top16[:, 0:8], enc[:])
            enc2 = bw.tile([128, C * 8], fp32, tag="enc2")
            nc.vector.match_replace(out=enc2[:], in_to_replace=top16[:, 0:8],
                                    in_values=enc[:], imm_value=-3.0e38)
            nc.vector.max(top16[:, 8:16], enc2[:])

            cols = bw.tile([128, TOPK], u32, tag="cols")
            nc.vector.tensor_scalar(cols[:], top16[:].bitcast(u32), 0x1FFF,
                                    None, op0=OP.bitwise_and)
            lvc = bw.tile([128, TOPK], fp32, tag="lvc")
            nc.vector.tensor_scalar(lvc[:].bitcast(u32), top16[:].bitcast(u32),
                                    0xFFFFE000, None, op0=OP.bitwise_and)
            nc.vector.tensor_copy(lvc[:, 0:1], m8[:, 0:1])

            den = bw.tile([128, 1], fp32, tag="den")
            nc.scalar.activation(logits[:].rearrange("p c w -> p (c w)"),
                                 logits[:].rearrange("p c w -> p (c w)"),
                                 AF.Exp, bias=negM[:], scale=1.0,
                                 accum_out=den[:])
            recip = bw.tile([128, 1], fp32, tag="recip")
            nc.vector.reciprocal(recip[:], den[:])

            evals = bw.tile([128, TOPK], fp32, tag="evals")
            nc.scalar.activation(evals[:], lvc[:], AF.Exp, bias=negM[:])
            vals = bw.tile([128, TOPK], fp32, tag="vals")
            nc.vector.tensor_scalar(vals[:], evals[:], recip[:], None,
                                    op0=OP.mult)

            col2 = bw.tile([128, TOPK, 2], i32, tag="col2")
            nc.vector.tensor_scalar(col2[:, :, 0:1], cols[:, :, None], 2.0,
                                    None, op0=OP.mult)
            nc.vector.tensor_scalar(col2[:, :, 1:2], cols[:, :, None], 2.0,
                                    1.0, op0=OP.mult, op1=OP.add)
            shifted = bw.tile([128, 2 * HCH, 2 * TOPK], i32, tag="shifted")
            nc.vector.tensor_tensor(
                shifted[:],
                col2[:].rearrange("p k two -> p (k two)")[:, None, :]
                    .to_broadcast([128, 2 * HCH, 2 * TOPK]),
                lsgrid[:].to_broadcast([128, 2 * HCH, 2 * TOPK]),
                op=OP.subtract)
            oob = bw.tile([128, 2 * HCH, 2 * TOPK], i32, tag="oob")
            nc.vector.tensor_scalar(oob[:], shifted[:], float(2 * CH), -32768.0,
                                    op0=OP.is_ge, op1=OP.mult)
            nc.vector.tensor_tensor(shifted[:], shifted[:], oob[:], op=OP.add)
            idx16 = bw.tile([128, 2 * HCH, 2 * TOPK], i16, tag="idx16")
            nc.vector.tensor_copy(idx16[:], shifted[:])

            data16 = vals[:].bitcast(u16)
            for h in range(2):
                outu = bsc.tile([128, 8192], u16, tag="outu")
                for c in range(HCH):
                    ne = 2 * CH if c < HCH - 1 else 8192 - 2 * CH * (HCH - 1)
                    nc.gpsimd.local_scatter(
                        outu[:, 2 * CH * c: 2 * CH * c + ne],
                        data16,
                        idx16[:, h * HCH + c, :],
                        channels=128, num_elems=ne, num_idxs=2 * TOPK)
                nc.sync.dma_start(ht_d[ts(t, 128), ts(h, 4096)],
                                  outu[:].bitcast(fp32))


def _host_prep(inputs):
    f = np.float32
    h = np.float16
    g0 = np.asarray(inputs["g0"], f); beta0 = np.asarray(inputs["beta0"], f)
    g1 = np.asarray(inputs["g1"], f); beta1 = np.asarray(inputs["beta1"], f)
    in_w = np.asarray(inputs["in_w"], f); in_b = np.asarray(inputs["in_b"], f)
    out_w = np.asarray(inputs["out_w"], f); out_b = np.asarray(inputs["out_b"], f)
    ew_w1 = np.asarray(inputs["ew_w1"], f); ew_b1 = np.asarray(inputs["ew_b1"], f)
    ew_w2 = np.asarray(inputs["ew_w2"], f); ew_b2 = np.asarray(inputs["ew_b2"], f)

    def bmat(b, noc):
        return np.ascontiguousarray(np.asarray(b, f).reshape(noc, 128).T)

    shared = {
        "wp0T": np.ascontiguousarray(np.asarray(inputs["w_p0"], f).T).astype(h),
        "bp0": bmat(inputs["b_p0"], 4),
        "wp1T": np.ascontiguousarray(np.asarray(inputs["w_p1"], f).T).astype(h),
        "bp1": bmat(inputs["b_p1"], 4),
        "qkvT0": np.ascontiguousarray((in_w * g0[None, :]).T).astype(h),
        "qkvb0": bmat(in_w @ beta0 + in_b, 12),
        "qkvT1": np.ascontiguousarray((in_w * g1[None, :]).T).astype(h),
        "qkvb1": bmat(in_w @ beta1 + in_b, 12),
        "outT": np.ascontiguousarray((0.5 * out_w).T).astype(h),
        "outb": bmat(out_b, 4),
        "ew1T": np.ascontiguousarray(ew_w1.T).astype(h),
        "ewb1": bmat(ew_b1, 2),
        "ew2T": np.ascontiguousarray(ew_w2.T).astype(h),
        "ewb2": np.asarray(ew_b2, f)[None, :],
    }
    sco = np.zeros((D, 8), f)
    for hh in range(8):
        sco[hh * 64:(hh + 1) * 64, hh] = 0.125
    shared["sco"] = sco.astype(h)
    shared["exp8"] = np.ascontiguousarray((sco.T != 0)).astype(h)

    x0T = np.ascontiguousarray(np.asarray(inputs["x0"], f).T).astype(h)
    x1T = np.ascontiguousarray(np.asarray(inputs["x1"], f).T).astype(h)
    maps = []
    for c in range(NCORES):
        m = dict(shared)
        m["x0T"] = np.ascontiguousarray(x0T[:, c * R:(c + 1) * R])
        m["x1T"] = np.ascontiguousarray(x1T[:, c * R:(c + 1) * R])
        maps.append(m)
    return maps


def kernel(**inputs):
    global _compiled, LAST_EXEC_NS
    import os
    from concourse.bass_utils import run_bass_kernel_spmd
    if _compiled is None:
        _compiled = _build()
    maps = _host_prep(inputs)
    trace = bool(os.environ.get("KERNEL_TRACE"))
    res = run_bass_kernel_spmd(_compiled, maps, core_ids=list(range(NCORES)),
                               trace=trace)
    LAST_EXEC_NS = res.exec_time_ns
    ht = np.concatenate([res.results[c]["ht"] for c in range(NCORES)], axis=0)
    H = np.ascontiguousarray(ht.T)
    ew = np.concatenate([res.results[c]["ew"][0] for c in range(NCORES)])
    return H, ew


# revision 19
# speedup vs baseline: 1.3639x; 1.1420x over previous
"""Trainium2 Bass kernel for nn_AblationAnomalyDetector (gnn_message_passing).

kernel(**inputs) -> (H [8192,8192] f32, ew [8192] f32)

8 NeuronCores, SPMD, node-dim sharded 1024 rows/core, fp16 matmuls:
  Phase A (transposed layout [feature-part, row-free]): proj -> LN (stats via
  ones-matmuls, biases via ACT per-partition bias APs) -> 2-token MHA ->
  fused^T [512,1024] fp16 + edge-weight MLP.
  AllGather fused^T (fp16) -> [8*512, 1024] shared.
  Phase B per 128-row tile: fp16 matmul logits [128, 8192] -> per-chunk DVE
  max/max_index -> bit-encoded candidate top-16 -> exact row max + softmax
  denominator via one in-place ACT exp pass (accum_out) -> 16 values
  scattered into u16 bit-planes via gpsimd local_scatter -> DMA to H^T.
"""
import numpy as np

N = 8192
D = 512
NCORES = 8
R = N // NCORES          # 1024 rows per core
NT = R // 128            # 8 row-tiles per core
C = 16                   # scan chunks per row
CW = N // C              # 512
TOPK = 16
CH = 1022                # fp32 cols per local_scatter chunk
HCH = 5                  # ls-chunks per half (4 full + tail)
EPS = 1e-5

_compiled = None
LAST_EXEC_NS = None


def _build():
    import concourse.bass as bass
    import concourse.tile as tile
    import concourse.mybir as mybir
    from concourse import bacc

    fp32 = mybir.dt.float32
    fp16 = mybir.dt.float16

    nc = bacc.Bacc("TRN2", target_bir_lowering=False, debug=False,
                   enable_asserts=True, num_devices=NCORES)

    def din16(name, shape):
        return nc.dram_tensor(name, shape, fp16, kind="ExternalInput").ap()

    def din32(name, shape):
        return nc.dram_tensor(name, shape, fp32, kind="ExternalInput").ap()

    env = {}
    env["x0T_d"] = din16("x0T", [D, R])
    env["x1T_d"] = din16("x1T", [D, R])
    env["wp0T_d"] = din16("wp0T", [D, D]); env["bp0_d"] = din32("bp0", [128, 4])
    env["wp1T_d"] = din16("wp1T", [D, D]); env["bp1_d"] = din32("bp1", [128, 4])
    env["qkvT0_d"] = din16("qkvT0", [D, 3 * D])
    env["qkvb0_d"] = din32("qkvb0", [128, 12])
    env["qkvT1_d"] = din16("qkvT1", [D, 3 * D])
    env["qkvb1_d"] = din32("qkvb1", [128, 12])
    env["outT_d"] = din16("outT", [D, D]); env["outb_d"] = din32("outb", [128, 4])
    env["ew1T_d"] = din16("ew1T", [D, D // 2])
    env["ewb1_d"] = din32("ewb1", [128, 2])
    env["ew2T_d"] = din16("ew2T", [D // 2, 1])
    env["ewb2_d"] = din32("ewb2", [1, 1])
    env["sco_d"] = din16("sco", [D, 8])
    env["exp8_d"] = din16("exp8", [8, D])

    env["ht_d"] = nc.dram_tensor("ht", [R, N], fp32, kind="ExternalOutput").ap()
    env["ew_d"] = nc.dram_tensor("ew", [1, R], fp32, kind="ExternalOutput").ap()

    env["agin"] = nc.dram_tensor("agin", [D, R], fp16)
    env["agout"] = nc.dram_tensor("agout", [NCORES * D, R], fp16,
                                  addr_space="Shared")

    with tile.TileContext(nc) as tc:
        with tc.tile_pool(name="pers", bufs=1) as pers:
            fused16 = pers.tile([128, 4, R], fp16, tag="fused16")
            env["fused16"] = fused16
            _phase_a(nc, tc, bass, mybir, env)
            nc.gpsimd.collective_compute(
                "AllGather", mybir.AluOpType.bypass,
                replica_groups=[list(range(NCORES))],
                ins=[env["agin"].ap()], outs=[env["agout"].ap()])
            _phase_b(nc, tc, bass, mybir, env)

    nc.compile()
    return nc


def _phase_a(nc, tc, bass, mybir, env):
    fp32 = mybir.dt.float32
    fp16 = mybir.dt.float16
    AF = mybir.ActivationFunctionType
    OP = mybir.AluOpType
    ts, ds = bass.ts, bass.ds
    fused16 = env["fused16"]

    with tc.tile_pool(name="aps", bufs=4, space="PSUM") as aps, \
         tc.tile_pool(name="stps", bufs=2, space="PSUM") as stps, \
         tc.tile_pool(name="p_ctx", bufs=1) as p_ctx:

        def load16(pool, dram_ap, kdim, fdim, tag):
            kc = max(kdim // 128, 1)
            p0 = min(kdim, 128)
            t = pool.tile([p0, kc, fdim], fp16, tag=tag)
            nc.sync.dma_start(t[:], dram_ap.rearrange("(c p) f -> p c f", p=p0))
            return t

        def loadb(pool, dram_ap, ncols, tag):
            t = pool.tile([dram_ap.shape[0], ncols], fp32, tag=tag)
            nc.sync.dma_start(t[:], dram_ap)
            return t

        ctx = p_ctx.tile([128, 4, R], fp16, tag="ctx")

        with tc.tile_pool(name="p_v", bufs=1) as p_v, \
             tc.tile_pool(name="p_qk", bufs=1) as p_qk, \
             tc.tile_pool(name="p_pn", bufs=1) as p_pn:
            # ---- stage 1: projections + LN (both modalities)
            pn = {}
            s1_cm = tc.tile_pool(name="s1", bufs=1)
            s1 = s1_cm.__enter__()
            ones4 = s1.tile([128, 4, 1], fp16, tag="ones4")
            nc.vector.memset(ones4[:], 1.0)
            for mi in (0, 1):
                xT = s1.tile([128, 4, R], fp16, tag=f"xT{mi}", name=f"xT{mi}")
                nc.sync.dma_start(
                    xT[:],
                    env[f"x{mi}T_d"].rearrange("(c p) f -> p c f", p=128))
                wpT = load16(s1, env[f"wp{mi}T_d"], D, D, f"wpT{mi}")
                bp = loadb(s1, env[f"bp{mi}_d"], 4, f"bp{mi}")
                prelu = s1.tile([128, 4, R], fp16, tag=f"prelu{mi}",
                                name=f"prelu{mi}")
                for oc in range(4):
                    for rc in range(2):
                        ps = aps.tile([128, 512], fp32, tag="mm")
                        for kc in range(4):
                            nc.tensor.matmul(ps[:], wpT[:, kc, ts(oc, 128)],
                                             xT[:, kc, ts(rc, 512)],
                                             start=(kc == 0), stop=(kc == 3))
                        nc.scalar.activation(prelu[:, oc, ts(rc, 512)], ps[:],
                                             AF.Relu, bias=bp[:, oc:oc + 1])
                psq = s1.tile([128, 4, R], fp16, tag="psq", name=f"psq{mi}")
                nc.scalar.activation(psq[:], prelu[:], AF.Square)
                s1v = s1.tile([1, 2, R], fp32, tag=f"s12{mi}", name=f"s12{mi}")
                for (si, src2) in ((0, prelu), (1, psq)):
                    for rc in range(2):
                        sps = stps.tile([1, 512], fp32, tag="st")
                        for kc in range(4):
                            nc.tensor.matmul(sps[:], ones4[:, kc, :],
                                             src2[:, kc, ts(rc, 512)],
                                             start=(kc == 0), stop=(kc == 3))
                        nc.vector.tensor_copy(s1v[:, si, ts(rc, 512)], sps[:])
                mean = s1.tile([1, R], fp32, tag=f"mean{mi}", name=f"mean{mi}")
                nc.vector.tensor_scalar(mean[:], s1v[:, 0, :], 1.0 / D,
                                        None, op0=OP.mult)
                var = s1.tile([1, R], fp32, tag=f"var{mi}", name=f"var{mi}")
                nc.vector.tensor_scalar(var[:], s1v[:, 1, :], 1.0 / D,
                                        None, op0=OP.mult)
                msq = s1.tile([1, R], fp32, tag=f"msq{mi}", name=f"msq{mi}")
                nc.vector.tensor_tensor(msq[:], mean[:], mean[:], op=OP.mult)
                nc.vector.tensor_scalar(msq[:], msq[:], float(EPS), None,
                                        op0=OP.subtract)
                nc.vector.tensor_tensor(var[:], var[:], msq[:], op=OP.subtract)
                std = s1.tile([1, R], fp32, tag=f"std{mi}", name=f"std{mi}")
                nc.scalar.activation(std[:], var[:], AF.Sqrt)
                istd = s1.tile([1, R], fp32, tag=f"istd{mi}", name=f"istd{mi}")
                nc.vector.reciprocal(istd[:], std[:])
                mean_b = s1.tile([128, R], fp32, tag="meanb",
                                 name=f"meanb{mi}")
                istd_b = s1.tile([128, R], fp32, tag="istdb",
                                 name=f"istdb{mi}")
                nc.gpsimd.partition_broadcast(mean_b[:], mean[:])
                nc.gpsimd.partition_broadcast(istd_b[:], istd[:])
                pnt = p_pn.tile([128, 4, R], fp16, tag=f"pn{mi}", name=f"pn{mi}")
                tmp = s1.tile([128, 4, R], fp16, tag="tmpn",
                              name=f"tmpn{mi}")
                nc.vector.tensor_tensor(
                    tmp[:], prelu[:],
                    mean_b[:, None, :].to_broadcast([128, 4, R]),
                    op=OP.subtract)
                nc.vector.tensor_tensor(
                    pnt[:], tmp[:],
                    istd_b[:, None, :].to_broadcast([128, 4, R]),
                    op=OP.mult)
                pn[mi] = pnt
            s1_cm.__exit__(None, None, None)

            # ---- stage 2: qkv
            qk = {}
            vv = {}
            for ti in (0, 1):
                with tc.tile_pool(name=f"s2w{ti}", bufs=1) as sw:
                    qkvT = load16(sw, env[f"qkvT{ti}_d"], D, 3 * D, "qkvT")
                    qkvb = loadb(sw, env[f"qkvb{ti}_d"], 12, "qkvb")
                    qkt = p_qk.tile([128, 8, R], fp16, tag=f"qk{ti}",
                                    name=f"qk{ti}")
                    vt = p_v.tile([128, 4, R], fp16, tag=f"v{ti}", name=f"v{ti}")
                    for oc in range(12):
                        dstt = qkt[:, oc, :] if oc < 8 else vt[:, oc - 8, :]
                        for rc in range(2):
                            ps = aps.tile([128, 512], fp32, tag="mm")
                            for kc in range(4):
                                nc.tensor.matmul(ps[:], qkvT[:, kc, ts(oc, 128)],
                                                 pn[ti][:, kc, ts(rc, 512)],
                                                 start=(kc == 0), stop=(kc == 3))
                            nc.scalar.activation(dstt[:, ts(rc, 512)], ps[:],
                                                 AF.Identity,
                                                 bias=qkvb[:, oc:oc + 1])
                    qk[ti] = qkt
                    vv[ti] = vt

            # ---- stage 3: scores + attention weights
            A = {}
            with tc.tile_pool(name="s3", bufs=1) as s3:
                sco = load16(s3, env["sco_d"], D, 8, "sco")
                s_t = {}
                for (qi, ki) in ((0, 0), (0, 1), (1, 0), (1, 1)):
                    e = s3.tile([128, 4, R], fp16, tag=f"ep{qi}{ki}",
                                name=f"ep{qi}{ki}")
                    nc.vector.tensor_tensor(e[:], qk[qi][:, 0:4, :],
                                            qk[ki][:, 4:8, :], op=OP.mult)
                    st_ = s3.tile([8, R], fp32, tag=f"s{qi}{ki}",
                                  name=f"s{qi}{ki}")
                    for rc in range(2):
                        sps = stps.tile([8, 512], fp32, tag="sc")
                        for kc in range(4):
                            nc.tensor.matmul(sps[:], sco[:, kc, :],
                                             e[:, kc, ts(rc, 512)],
                                             start=(kc == 0), stop=(kc == 3))
                        nc.vector.tensor_copy(st_[:, ts(rc, 512)], sps[:])
                    s_t[(qi, ki)] = st_
                sig = {}
                for qi in (0, 1):
                    dd = s3.tile([8, R], fp32, tag=f"d{qi}", name=f"d{qi}")
                    nc.vector.tensor_tensor(dd[:], s_t[(qi, 0)][:],
                                            s_t[(qi, 1)][:], op=OP.subtract)
                    sg = s3.tile([8, R], fp32, tag=f"sg{qi}", name=f"sg{qi}")
                    nc.scalar.activation(sg[:], dd[:], AF.Sigmoid)
                    sig[qi] = sg
                A0 = p_v.tile([8, R], fp16, tag="A0")
                nc.vector.tensor_tensor(A0[:], sig[0][:], sig[1][:], op=OP.add)
                A1 = p_v.tile([8, R], fp16, tag="A1")
                nc.vector.tensor_scalar(A1[:], A0[:], -1.0, 2.0,
                                        op0=OP.mult, op1=OP.add)
                A[0], A[1] = A0, A1

            # ---- stage 4: ctx
            with tc.tile_pool(name="s4c", bufs=1) as s4c:
                exp8 = load16(s4c, env["exp8_d"], 8, D, "exp8")
                ctmp = s4c.tile([128, 4, R], fp16, tag="ctmp")
                ctmp2 = s4c.tile([128, 4, R], fp16, tag="ctmp2")
                for (ai, dstc) in ((0, ctmp), (1, ctmp2)):
                    Ae = s4c.tile([128, 4, R], fp16, tag=f"Ae{ai}",
                                  name=f"Ae{ai}")
                    for oc in range(4):
                        for rc in range(2):
                            ps = aps.tile([128, 512], fp32, tag="mm")
                            nc.tensor.matmul(ps[:], exp8[:, 0, ts(oc, 128)],
                                             A[ai][:, ts(rc, 512)],
                                             start=True, stop=True)
                            nc.scalar.activation(Ae[:, oc, ts(rc, 512)], ps[:],
                                                 AF.Copy)
                    nc.vector.tensor_tensor(dstc[:], Ae[:],
                                            vv[ai][:, 0:4, :], op=OP.mult)
                nc.vector.tensor_tensor(ctx[:], ctmp[:], ctmp2[:], op=OP.add)

        # ---- stage 5: fused + edge weights
        with tc.tile_pool(name="s5", bufs=1) as s5:
            outT = load16(s5, env["outT_d"], D, D, "outT")
            outb = loadb(s5, env["outb_d"], 4, "outb")
            for oc in range(4):
                for rc in range(2):
                    ps = aps.tile([128, 512], fp32, tag="mm")
                    for kc in range(4):
                        nc.tensor.matmul(ps[:], outT[:, kc, ts(oc, 128)],
                                         ctx[:, kc, ts(rc, 512)],
                                         start=(kc == 0), stop=(kc == 3))
                    nc.scalar.activation(fused16[:, oc, ts(rc, 512)], ps[:],
                                         AF.Identity, bias=outb[:, oc:oc + 1])
            nc.sync.dma_start(
                env["agin"].ap().rearrange("(c p) r -> p c r", p=128),
                fused16[:])

            ew1T = load16(s5, env["ew1T_d"], D, D // 2, "ew1T")
            ewb1 = loadb(s5, env["ewb1_d"], 2, "ewb1")
            ew2T = load16(s5, env["ew2T_d"], D // 2, 1, "ew2T")
            ewb2 = loadb(s5, env["ewb2_d"], 1, "ewb2")
            hid = s5.tile([128, 2, R], fp16, tag="hid")
            for oc in range(2):
                for rc in range(2):
                    ps = aps.tile([128, 512], fp32, tag="mm")
                    for kc in range(4):
                        nc.tensor.matmul(ps[:], ew1T[:, kc, ts(oc, 128)],
                                         fused16[:, kc, ts(rc, 512)],
                                         start=(kc == 0), stop=(kc == 3))
                    nc.scalar.activation(hid[:, oc, ts(rc, 512)], ps[:], AF.Relu,
                                         bias=ewb1[:, oc:oc + 1])
            ew_sb = s5.tile([1, R], fp32, tag="ew_sb")
            for rc in range(2):
                sps = stps.tile([1, 512], fp32, tag="st")
                for kc in range(2):
                    nc.tensor.matmul(sps[:], ew2T[:, kc, :],
                                     hid[:, kc, ts(rc, 512)],
                                     start=(kc == 0), stop=(kc == 1))
                nc.scalar.activation(ew_sb[:, ts(rc, 512)], sps[:], AF.Sigmoid,
                                     bias=ewb2[:, 0:1])
            ewm = s5.tile([1, R], fp32, tag="ewm")
            nc.vector.tensor_scalar(ewm[:], ew_sb[:], 1e-8, None, op0=OP.max)
            nc.sync.dma_start(env["ew_d"], ewm[:])


def _phase_b(nc, tc, bass, mybir, env):
    fp32 = mybir.dt.float32
    fp16 = mybir.dt.float16
    u32 = mybir.dt.uint32
    i32 = mybir.dt.int32
    i16 = mybir.dt.int16
    u16 = mybir.dt.uint16
    AF = mybir.ActivationFunctionType
    OP = mybir.AluOpType
    ts, ds = bass.ts, bass.ds
    fused16 = env["fused16"]
    agout = env["agout"]
    ht_d = env["ht_d"]

    with tc.tile_pool(name="brhs", bufs=1) as brhs, \
         tc.tile_pool(name="blog", bufs=2) as blog, \
         tc.tile_pool(name="bsc", bufs=2) as bsc, \
         tc.tile_pool(name="bw", bufs=2) as bw, \
         tc.tile_pool(name="bps", bufs=8, space="PSUM") as bps:

        rhs = brhs.tile([128, 4, NCORES, R], fp16, tag="rhs")
        for s in range(NCORES):
            nc.sync.dma_start(
                rhs[:, :, s, :],
                agout.ap()[s * D:(s + 1) * D, :].rearrange(
                    "(c p) r -> p c r", p=128))

        lsgrid = brhs.tile([128, 2 * HCH, 1], i32, tag="lsgrid")
        nc.gpsimd.iota(lsgrid[:, 0:HCH, :], pattern=[[2 * CH, HCH], [0, 1]],
                       base=0, channel_multiplier=0)
        nc.gpsimd.iota(lsgrid[:, HCH:2 * HCH, :], pattern=[[2 * CH, HCH], [0, 1]],
                       base=2 * 4096, channel_multiplier=0)
        cgrid = brhs.tile([128, C, 1], u32, tag="cgrid")
        nc.gpsimd.iota(cgrid[:], pattern=[[CW, C], [0, 1]], base=0,
                       channel_multiplier=0)

        for t in range(NT):
            logits = blog.tile([128, C, CW], fp32, tag="logits")
            for g in range(2):
                pss = [bps.tile([128, 512], fp32, tag="pb", name=f"pb{j}") for j in range(8)]
                for kc in range(4):
                    for j in range(8):
                        cb = g * 8 + j
                        s, half = cb // 2, cb % 2
                        nc.tensor.matmul(pss[j][:],
                                         fused16[:, kc, ds(t * 128, 128)],
                                         rhs[:, kc, s, ts(half, 512)],
                                         start=(kc == 0), stop=(kc == 3))
                for j in range(8):
                    cb = g * 8 + j
                    nc.scalar.activation(logits[:, cb, :], pss[j][:], AF.Copy)

            cmax = bw.tile([128, C, 8], fp32, tag="cmax")
            cidx = bw.tile([128, C, 8], u32, tag="cidx")
            for cb in range(C):
                nc.vector.max(cmax[:, cb, :], logits[:, cb, :])
                nc.vector.max_index(cidx[:, cb, :], cmax[:, cb, :],
                                    logits[:, cb, :])
            m8 = bw.tile([128, 8], fp32, tag="m8")
            nc.vector.max(m8[:], cmax[:].rearrange("p c e -> p (c e)"))
            negM = bw.tile([128, 1], fp32, tag="negM")
            nc.vector.tensor_scalar(negM[:], m8[:, 0:1], -1.0, None, op0=OP.mult)

            colf = bw.tile([128, C, 8], u32, tag="colf")
            nc.vector.tensor_tensor(colf[:], cidx[:],
                                    cgrid[:].to_broadcast([128, C, 8]), op=OP.add)
            enc = bw.tile([128, C * 8], fp32, tag="enc")
            nc.vector.tensor_scalar(enc[:].bitcast(u32),
                                    cmax[:].rearrange("p c e -> p (c e)").bitcast(u32),
                                    0xFFFFE000, None, op0=OP.bitwise_and)
            nc.vector.tensor_tensor(enc[:].bitcast(u32), enc[:].bitcast(u32),
                                    colf[:].rearrange("p c e -> p (c e)"),
                                    op=OP.bitwise_or)
            top16 = bw.tile([128, TOPK], fp32, tag="top16")
            nc.vector.max(top16[:, 0:8], enc[:])
            enc2 = bw.tile([128, C * 8], fp32, tag="enc2")
            nc.vector.match_replace(out=enc2[:], in_to_replace=top16[:, 0:8],
                                    in_values=enc[:], imm_value=-3.0e38)
            nc.vector.max(top16[:, 8:16], enc2[:])

            cols = bw.tile([128, TOPK], u32, tag="cols")
            nc.vector.tensor_scalar(cols[:], top16[:].bitcast(u32), 0x1FFF,
                                    None, op0=OP.bitwise_and)
            lvc = bw.tile([128, TOPK], fp32, tag="lvc")
            nc.vector.tensor_scalar(lvc[:].bitcast(u32), top16[:].bitcast(u32),
                                    0xFFFFE000, None, op0=OP.bitwise_and)
            nc.vector.tensor_copy(lvc[:, 0:1], m8[:, 0:1])

            den = bw.tile([128, 1], fp32, tag="den")
            nc.scalar.activation(logits[:].rearrange("p c w -> p (c w)"),
                                 logits[:].rearrange("p c w -> p (c w)"),
                                 AF.Exp, bias=negM[:], scale=1.0,
                                 accum_out=den[:])
            recip = bw.tile([128, 1], fp32, tag="recip")
            nc.vector.reciprocal(recip[:], den[:])

            evals = bw.tile([128, TOPK], fp32, tag="evals")
            nc.scalar.activation(evals[:], lvc[:], AF.Exp, bias=negM[:])
            vals = bw.tile([128, TOPK], fp32, tag="vals")
            nc.vector.tensor_scalar(vals[:], evals[:], recip[:], None,
                                    op0=OP.mult)

            col2 = bw.tile([128, TOPK, 2], i32, tag="col2")
            nc.vector.tensor_scalar(col2[:, :, 0:1], cols[:, :, None], 2.0,
                                    None, op0=OP.mult)
            nc.vector.tensor_scalar(col2[:, :, 1:2], cols[:, :, None], 2.0,
                                    1.0, op0=OP.mult, op1=OP.add)
            shifted = bw.tile([128, 2 * HCH, 2 * TOPK], i32, tag="shifted")
            nc.vector.tensor_tensor(
                shifted[:],
                col2[:].rearrange("p k two -> p (k two)")[:, None, :]
                    .to_broadcast([128, 2 * HCH, 2 * TOPK]),
                lsgrid[:].to_broadcast([128, 2 * HCH, 2 * TOPK]),
                op=OP.subtract)
            oob = bw.tile([128, 2 * HCH, 2 * TOPK], i32, tag="oob")
            nc.vector.tensor_scalar(oob[:], shifted[:], float(2 * CH), -32768.0,
                                    op0=OP.is_ge, op1=OP.mult)
            nc.vector.tensor_tensor(shifted[:], shifted[:], oob[:], op=OP.add)
            idx16 = bw.tile([128, 2 * HCH, 2 * TOPK], i16, tag="idx16")
            nc.vector.tensor_copy(idx16[:], shifted[:])

            data16 = vals[:].bitcast(u16)
            for h in range(2):
                outu = bsc.tile([128, 8192], u16, tag="outu")
                for c in range(HCH):
                    ne = 2 * CH if c < HCH - 1 else 8192 - 2 * CH * (HCH - 1)
                    nc.gpsimd.local_scatter(
                        outu[:, 2 * CH * c: 2 * CH * c + ne],
                        data16,
                        idx16[:, h * HCH + c, :],
                        channels=128, num_elems=ne, num_idxs=2 * TOPK)
                nc.sync.dma_start(ht_d[ts(t, 128), ts(h, 4096)],
                                  outu[:].bitcast(fp32))


def _host_prep(inputs):
    f = np.float32
    h = np.float16
    g0 = np.asarray(inputs["g0"], f); beta0 = np.asarray(inputs["beta0"], f)
    g1 = np.asarray(inputs["g1"], f); beta1 = np.asarray(inputs["beta1"], f)
    in_w = np.asarray(inputs["in_w"], f); in_b = np.asarray(inputs["in_b"], f)
    out_w = np.asarray(inputs["out_w"], f); out_b = np.asarray(inputs["out_b"], f)
    ew_w1 = np.asarray(inputs["ew_w1"], f); ew_b1 = np.asarray(inputs["ew_b1"], f)
    ew_w2 = np.asarray(inputs["ew_w2"], f); ew_b2 = np.asarray(inputs["ew_b2"], f)

    def bmat(b, noc):
        return np.ascontiguousarray(np.asarray(b, f).reshape(noc, 128).T)

    shared = {
        "wp0T": np.ascontiguousarray(np.asarray(inputs["w_p0"], f).T).astype(h),
        "bp0": bmat(inputs["b_p0"], 4),
        "wp1T": np.ascontiguousarray(np.asarray(inputs["w_p1"], f).T).astype(h),
        "bp1": bmat(inputs["b_p1"], 4),
        "qkvT0": np.ascontiguousarray((in_w * g0[None, :]).T).astype(h),
        "qkvb0": bmat(in_w @ beta0 + in_b, 12),
        "qkvT1": np.ascontiguousarray((in_w * g1[None, :]).T).astype(h),
        "qkvb1": bmat(in_w @ beta1 + in_b, 12),
        "outT": np.ascontiguousarray((0.5 * out_w).T).astype(h),
        "outb": bmat(out_b, 4),
        "ew1T": np.ascontiguousarray(ew_w1.T).astype(h),
        "ewb1": bmat(ew_b1, 2),
        "ew2T": np.ascontiguousarray(ew_w2.T).astype(h),
        "ewb2": np.asarray(ew_b2, f)[None, :],
    }
    sco = np.zeros((D, 8), f)
    for hh in range(8):
        sco[hh * 64:(hh + 1) * 64, hh] = 0.125
    shared["sco"] = sco.astype(h)
    shared["exp8"] = np.ascontiguousarray((sco.T != 0)).astype(h)

    x0T = np.ascontiguousarray(np.asarray(inputs["x0"], f).T).astype(h)
    x1T = np.ascontiguousarray(np.asarray(inputs["x1"], f).T).astype(h)
    maps = []
    for c in range(NCORES):
        m = dict(shared)
        m["x0T"] = np.ascontiguousarray(x0T[:, c * R:(c + 1) * R])
        m["x1T"] = np.ascontiguousarray(x1T[:, c * R:(c + 1) * R])
        maps.append(m)
    return maps


def kernel(**inputs):
    global _compiled, LAST_EXEC_NS
    import os
    from concourse.bass_utils import run_bass_kernel_spmd
    if _compiled is None:
        _compiled = _build()
    maps = _host_prep(inputs)
    trace = bool(os.environ.get("KERNEL_TRACE"))
    res = run_bass_kernel_spmd(_compiled, maps, core_ids=list(range(NCORES)),
                               trace=trace)
    LAST_EXEC_NS = res.exec_time_ns
    ht = np.concatenate([res.results[c]["ht"] for c in range(NCORES)], axis=0)
    H = np.ascontiguousarray(ht.T)
    ew = np.concatenate([res.results[c]["ew"][0] for c in range(NCORES)])
    return H, ew
